# revision 1
# baseline (speedup 1.0000x reference)
"""BarrierNet Trainium2 kernel: MLP + batched 2-var QP (active-set enumeration).

Self-contained: shards B=262144 samples across 8 NeuronCores (data parallel),
runs a Bass/Tile kernel per core, gathers the full output.

Layout per core (SHARD = 32768 samples):
  - MLP runs in 64 tiles of 512 samples, feature-major on the PE
    (fp32r matmuls; x transposed via PE; relu+bias fused in ACT epilogues).
  - L3 outputs (p1, p2, z1, z2) are staged as rows to DRAM, then reloaded as
    sample-major "planes" [128, 256] (sample s = p*256 + c).
  - The QP stage (constraint build + 11-candidate enumeration + argmin) runs
    on fp32 planes across DVE / GPSIMD / ACT.
"""
import numpy as np

import concourse.bass as bass
import concourse.bacc as bacc
import concourse.tile as tile
from concourse import mybir
from concourse.bass_utils import run_bass_kernel_spmd

F32 = mybir.dt.float32
F32R = mybir.dt.float32r
U8 = mybir.dt.uint8
Alu = mybir.AluOpType
Act = mybir.ActivationFunctionType

NCORES = 8
B = 262144
SHARD = B // NCORES            # 32768
PC = SHARD // 128              # 256  (plane free dim)
TILE_N = 512
NT = SHARD // TILE_N           # 64
NF, H1, H2 = 8, 256, 128

# Constants of the nn.Module (not inputs)
STATIC_OBS = np.array([[8.0, -8.0, 1.0], [-9.0, 7.0, 1.0], [10.0, 10.0, 1.5]],
                      np.float32)
AGENT_R, SAFETY = np.float32(0.5), np.float32(0.1)
TOL = 1e-6

_NC_CACHE = {}


def _f(x):
    return float(np.float32(x))


# ---------------------------------------------------------------------------
# Plane-op emission helpers: round-robin 2-input ops over DVE/GPSIMD.
class Emit:
    def __init__(self, nc, pool):
        self.nc = nc
        self.pool = pool
        self._rr = 0
        self._tmp = 0

    def plane(self, tag):
        return self.pool.tile([128, PC], F32, tag=tag, name=tag)

    def tmp(self):
        self._tmp += 1
        t = f"tmp{self._tmp % 9}"
        return self.pool.tile([128, PC], F32, tag=t, name=t, bufs=2)

    def mask(self, tag):
        return self.pool.tile([128, PC], U8, tag=tag, name=tag, bufs=2)

    _CMP = (Alu.is_ge, Alu.is_gt, Alu.is_le, Alu.is_lt)

    def _eng(self, force=None):
        if force is not None:
            return force
        self._rr += 1
        # 1:1 vector:gpsimd for 2-input ops
        return self.nc.gpsimd if self._rr % 2 == 0 else self.nc.vector

    _POOL_OK = (Alu.add, Alu.subtract, Alu.mult)

    def tt(self, out, a, b, op, eng=None):
        if op not in self._POOL_OK:
            eng = self.nc.vector
        self._eng(eng).tensor_tensor(out, a, b, op)
        return out

    def stt(self, out, a, s, b, op0, op1, eng=None):
        # TensorScalarPtr is not supported on Pool
        self.nc.vector.scalar_tensor_tensor(out, a, _f(s), b, op0, op1)
        return out

    def ts(self, out, a, s1, s2, op0, op1=None, eng=None):
        e = self.nc.vector if eng is None else eng
        s1 = s1 if isinstance(s1, bass.AP) else _f(s1)
        if s2 is None:
            e.tensor_scalar(out, a, s1, None, op0)
        else:
            s2 = s2 if isinstance(s2, bass.AP) else _f(s2)
            e.tensor_scalar(out, a, s1, s2, op0, op1)
        return out

    def sq(self, out, a):
        # square via Pool TT (offloads the scalar engine)
        self.nc.gpsimd.tensor_tensor(out, a, a, Alu.mult)
        return out

    def act(self, out, a, func, bias=0.0, scale=1.0):
        self.nc.scalar.activation(out, a, func, bias=bias, scale=scale)
        return out


def _build_nc(zero_bias=False):
    nc = bacc.Bacc("TRN2", target_bir_lowering=False, debug=False,
                   num_devices=NCORES)

    x_d = nc.dram_tensor("x", [SHARD, NF], F32, kind="ExternalInput")
    w1t_d = nc.dram_tensor("w1t", [NF, H1], F32, kind="ExternalInput")
    wcat_d = nc.dram_tensor("wcat", [128, 512], F32, kind="ExternalInput")
    w3t_d = nc.dram_tensor("w3t", [128, 8], F32, kind="ExternalInput")
    ident_d = nc.dram_tensor("ident", [128, 128], F32, kind="ExternalInput")
    consts_d = nc.dram_tensor("consts", [128, 16], F32, kind="ExternalInput")
    out_d = nc.dram_tensor("out", [SHARD, 2], F32, kind="ExternalOutput")

    with tile.TileContext(nc) as tc:
        with tc.tile_pool(name="wpool", bufs=1) as wp, \
             tc.tile_pool(name="mlp", bufs=2) as mp, \
             tc.tile_pool(name="planes", bufs=1) as pp, \
             tc.tile_pool(name="tmps", bufs=2) as tp, \
             tc.tile_pool(name="psum", bufs=1, space="PSUM") as ps, \
             tc.tile_pool(name="psum2", bufs=1, space="PSUM") as ps2, \
             tc.tile_pool(name="dram", bufs=1, space="DRAM") as dp:

            em = Emit(nc, pp)
            em_t = Emit(nc, tp)

            # ---------------- constants / weights ----------------
            w1t_sb = wp.tile([NF, H1], F32, tag="w1t32")
            wcat_sb = wp.tile([128, 2 * 256], F32, tag="wcat32")
            w3t_sb = wp.tile([128, 2 * 4], F32, tag="w3t32")
            ident_t = wp.tile([128, 128], F32, tag="ident")
            cs = wp.tile([128, 16], F32, tag="consts")
            nc.sync.dma_start(out=w1t_sb[:], in_=w1t_d.ap())
            nc.sync.dma_start(out=wcat_sb[:], in_=wcat_d.ap())
            nc.sync.dma_start(out=w3t_sb[:], in_=w3t_d.ap())
            nc.sync.dma_start(out=ident_t[:], in_=ident_d.ap())
            nc.sync.dma_start(out=cs[:], in_=consts_d.ap())

            # round weights to f32r once
            w1t_r = wp.tile([NF, H1], F32R, tag="w1tr")
            wcat_r = wp.tile([128, 2 * 256], F32R, tag="wcatr")
            w3t_r = wp.tile([128, 2 * 4], F32R, tag="w3tr")
            nc.scalar.activation(w1t_r[:], w1t_sb[:], Act.Copy)
            nc.scalar.activation(wcat_r[:], wcat_sb[:], Act.Copy)
            nc.scalar.activation(w3t_r[:], w3t_sb[:], Act.Copy)

            # xbig: planes view of x  [128, (c=256, f=8)]
            xbig = wp.tile([128, PC * NF], F32, tag="xbig")
            nc.sync.dma_start(
                out=xbig[:], in_=x_d.ap().rearrange("(p c) f -> p (c f)", p=128))
            xb3 = xbig[:].rearrange("p (c f) -> p c f", f=NF)

            def xf(i):  # feature-i plane view, strided
                return xb3[:, :, i]

            # z-row staging in DRAM
            z3_dram = dp.tile([4, SHARD], F32, tag="z3d")

            # ---------------- MLP over 64 tiles ----------------
            for t in range(NT):
                xc = mp.tile([128, 32], F32, tag="xc")
                xc3 = xc[:].rearrange("p (u f) -> p u f", u=4)
                nc.sync.dma_start(
                    out=xc3,
                    in_=x_d.ap()[t * TILE_N:(t + 1) * TILE_N, :]
                    .rearrange("(u p) f -> p u f", p=128))

                xT_ps = ps2.tile([NF, TILE_N], F32, tag="xTps", bufs=2)
                for u in range(4):
                    nc.tensor.transpose(xT_ps[:, u * 128:(u + 1) * 128],
                                        xc3[:, u, :], ident_t[:])
                xT_r = mp.tile([NF, TILE_N], F32R, tag="xTr")
                nc.scalar.activation(xT_r[:], xT_ps[:], Act.Copy)

                # L1: h1[mc] = relu(W1[mc] @ xT + b1[mc])  (K=8)
                # both m-chunks land in one 2-bank PSUM tile; single fused
                # bias+relu+f32r-round epilogue [128, 1024] on ACT
                h1r = mp.tile([128, 2 * TILE_N], F32R, tag="h1r")
                h1_ps = ps.tile([128, 2 * TILE_N], F32, tag="h1ps", bufs=1)
                for mc in range(2):
                    nc.tensor.matmul(
                        h1_ps[:, mc * TILE_N:(mc + 1) * TILE_N],
                        w1t_r[:, mc * 128:(mc + 1) * 128],
                        xT_r[:], start=True, stop=True)
                if zero_bias:
                    nc.scalar.activation(h1r[:], h1_ps[:], Act.Relu)
                else:
                    nc.scalar.activation(h1r[:, 0:TILE_N], h1_ps[:, 0:TILE_N],
                                         Act.Relu, bias=cs[:, 0:1], scale=1.0)
                    nc.vector.tensor_scalar(h1r[:, TILE_N:], h1_ps[:, TILE_N:],
                                            cs[:, 1:2], 0.0, Alu.add, Alu.max)

                # L2: h2[mo] = relu(sum_kc WcatT[kc][:,mo] @ h1r[kc] + bcat[mo])
                h2r = mp.tile([128, 2 * TILE_N], F32R, tag="h2r")
                h2_ps = ps.tile([128, 2 * TILE_N], F32, tag="h2ps")
                for mo in range(2):
                    for kc in range(2):
                        nc.tensor.matmul(
                            h2_ps[:, mo * TILE_N:(mo + 1) * TILE_N],
                            wcat_r[:, kc * 256 + mo * 128:kc * 256 + (mo + 1) * 128],
                            h1r[:, kc * TILE_N:(kc + 1) * TILE_N],
                            start=(kc == 0), stop=(kc == 1))
                if zero_bias:
                    nc.vector.tensor_scalar(h2r[:], h2_ps[:], 0.0, None,
                                            Alu.max)
                else:
                    nc.scalar.activation(h2r[:, 0:TILE_N], h2_ps[:, 0:TILE_N],
                                         Act.Relu, bias=cs[:, 2:3], scale=1.0)
                    nc.vector.tensor_scalar(h2r[:, TILE_N:], h2_ps[:, TILE_N:],
                                            cs[:, 3:4], 0.0, Alu.add, Alu.max)

                # L3: z3 = W3blk @ [x21; x22] + b3   ([4, 512])
                z3_ps = ps2.tile([4, TILE_N], F32, tag="z3ps", bufs=2)
                for kc in range(2):
                    nc.tensor.matmul(z3_ps[:],
                                     w3t_r[:, kc * 4:(kc + 1) * 4],
                                     h2r[:, kc * TILE_N:(kc + 1) * TILE_N],
                                     start=(kc == 0), stop=(kc == 1))
                # raw z3 -> SBUF -> DRAM; b3 is folded into the plane reload
                z3_sb = mp.tile([4, TILE_N], F32, tag="z3sb")
                if t % 2 == 0:
                    nc.scalar.activation(z3_sb[:], z3_ps[:], Act.Copy)
                else:
                    nc.vector.tensor_copy(z3_sb[:], z3_ps[:])
                nc.sync.dma_start(
                    out=z3_dram[:, t * TILE_N:(t + 1) * TILE_N], in_=z3_sb[:])

            # ============ QP stage on sample-major planes ============
            # Per-k quantities in wide tiles [128, 4*PC] (k-major slabs);
            # per-pair in [128, 6*PC]. Step-0 broadcast APs (DVE only) map
            # per-sample planes across slabs. Wide scratch buffers are
            # allocated once and reused with explicit live ranges.
            W4, W6 = 4 * PC, 6 * PC

            # --- greedy engine balancer (ns cost estimates) ---
            eng_load = {"v": 0.0, "g": 0.0, "a": 0.0}

            def _pick(costs):
                e = min(costs, key=lambda k: eng_load[k] + costs[k])
                eng_load[e] += costs[e]
                return e

            def _fd(ap):
                return ap.free_size()

            def wtt(out, a, b, op, bcast=False):
                fd = _fd(out)
                if bcast or op not in Emit._POOL_OK:
                    eng_load["v"] += 157 + fd / 0.96
                    nc.vector.tensor_tensor(out, a, b, op)
                    return out
                e = _pick({"v": (157 + fd / 0.96) * 1.55,
                           "g": 220 + fd * 2.17})
                (nc.vector if e == "v" else nc.gpsimd).tensor_tensor(out, a, b, op)
                return out

            def wsq(out, a):
                fd = _fd(out)
                e = _pick({"v": (157 + fd / 0.96) * 1.55,
                           "g": 220 + fd * 2.17,
                           "a": 187 + fd / 1.2})
                if e == "a":
                    nc.scalar.activation(out, a, Act.Square)
                else:
                    (nc.vector if e == "v" else nc.gpsimd).tensor_tensor(
                        out, a, a, Alu.mult)
                return out

            def wts(out, a, s1, s2, op0, op1=None):
                eng_load["v"] += 157 + _fd(out) / 1.92
                s1 = s1 if isinstance(s1, bass.AP) else _f(s1)
                if s2 is None:
                    nc.vector.tensor_scalar(out, a, s1, None, op0)
                else:
                    s2 = s2 if isinstance(s2, bass.AP) else _f(s2)
                    nc.vector.tensor_scalar(out, a, s1, s2, op0, op1)
                return out

            def wstt(out, a, s, b, op0, op1):
                eng_load["v"] += 157 + _fd(out) / 0.96
                nc.vector.scalar_tensor_tensor(out, a, _f(s), b, op0, op1)
                return out

            def wact(out, a, func, bias=0.0, scale=1.0):
                eng_load["a"] += 187 + _fd(out) / 1.2
                nc.scalar.activation(out, a, func, bias=bias, scale=scale)
                return out

            def wrecip(out, a):
                eng_load["v"] += 157 + _fd(out) / 0.96
                nc.vector.reciprocal_approx_fast(out, a)
                return out

            def named(tag, n=4):
                return pp.tile([128, n * PC], F32, tag=tag, name=tag)

            def slab(w, i):
                return w[:, i * PC:(i + 1) * PC]

            def bc(plane_ap, n):
                return plane_ap.rearrange(
                    "p (o c) -> p o c", o=1).to_broadcast((128, n, PC))

            def w3(w, n=4):
                return w[:].rearrange("p (o c) -> p o c", o=n)

            # explicit wide scratch buffers
            wa = [named(f"wa{i}") for i in range(6)]          # [128, 4*PC]
            wb = [named(f"wb{i}", 6) for i in range(6)]       # [128, 6*PC]

            # ---------------- planes: reload z rows ----------------
            # b3 = [b31; b32] folding: p = z + b31; sigmoid bias adds b32.
            zpl = []
            for i in range(4):
                z = em_t.tmp()
                nc.sync.dma_start(
                    out=z[:],
                    in_=z3_dram[i, :].rearrange("(p c) -> p c", p=128))
                zpl.append(z)
            zr1, zr2, zs1, zs2 = (z[:] for z in zpl)
            p1t = em.plane("p1")
            wact(p1t[:], zr1, Act.Identity, bias=cs[:, 13:14])
            p2t = em.plane("p2")
            wact(p2t[:], zr2, Act.Identity, bias=cs[:, 14:15])
            p1, p2 = p1t[:], p2t[:]

            # ---------------- geometry (x-derived), wide over k ----------
            mu = [0.0, 0.0, 0.0, 1.0, 6.0, 6.0]
            sg = [1.0, 1.0, 0.5, 0.3, 1.0, 1.0]
            rtot = np.concatenate(
                [AGENT_R + STATIC_OBS[:, 2] + SAFETY,
                 np.array([2 * AGENT_R + SAFETY], np.float32)]).astype(np.float32)
            r2 = (rtot * rtot).astype(np.float32)

            DXW, DYW = wa[0], wa[1]
            for k in range(3):
                wts(slab(DXW, k), xf(0), sg[0], cs[:, 5 + k:6 + k],
                    Alu.mult, Alu.add)
                wts(slab(DYW, k), xf(1), sg[1], cs[:, 8 + k:9 + k],
                    Alu.mult, Alu.add)
            oxo = em_t.tmp()
            wts(oxo[:], xf(4), sg[4], cs[:, 11:12], Alu.mult, Alu.add)
            oyo = em_t.tmp()
            wts(oyo[:], xf(5), sg[5], cs[:, 12:13], Alu.mult, Alu.add)
            wstt(slab(DXW, 3), xf(0), sg[0], oxo[:], Alu.mult, Alu.subtract)
            wstt(slab(DYW, 3), xf(1), sg[1], oyo[:], Alu.mult, Alu.subtract)

            st = em.plane("st")
            wact(st[:], xf(2), Act.Sin, bias=0.0, scale=sg[2])
            sh = em_t.tmp()
            wact(sh[:], xf(2), Act.Sin, bias=0.0, scale=sg[2] * 0.5)
            sh2 = em_t.tmp()
            wsq(sh2[:], sh[:])
            ct = em.plane("ct")
            wts(ct[:], sh2[:], -2.0, 1.0, Alu.mult, Alu.add)

            v = em.plane("v")
            wts(v[:], xf(3), sg[3], float(mu[3]), Alu.mult, Alu.add)
            vst = em.plane("vst")
            wtt(vst[:], v[:], st[:], Alu.mult)
            vct = em.plane("vct")
            wtt(vct[:], v[:], ct[:], Alu.mult)
            h0 = em.plane("H0")
            wstt(h0[:], v[:], 2.0, v[:], Alu.mult, Alu.mult)   # 2*v^2

            sg1 = em.plane("sg1")
            wact(sg1[:], zs1, Act.Sigmoid, bias=cs[:, 15:16])
            sg2p = em.plane("sg2")
            wact(sg2p[:], zs2, Act.Sigmoid, bias=cs[:, 4:5])
            sab = em.plane("sab")
            wtt(sab[:], sg1[:], sg2p[:], Alu.add)
            smm = em.plane("smm")
            wtt(smm[:], sg1[:], sg2p[:], Alu.mult)

            DX3, DY3 = w3(DXW), w3(DYW)
            G1W, G2W = named("G1W"), named("G2W")
            HQW, HTW = named("HQW"), named("HTW")
            QW, M0W = named("QW"), named("M0W")
            A, Bv, C, D = wa[2], wa[3], wa[4], wa[5]

            # materialized broadcast wides (enable Pool for the products)
            VCT4, VST4 = named("VCT4"), named("VST4")
            P1W, P2W = named("P1W"), named("P2W")
            wts(w3(VCT4), bc(vct[:], 4), 1.0, None, Alu.mult)
            wts(w3(VST4), bc(vst[:], 4), 1.0, None, Alu.mult)
            wts(w3(P1W), bc(p1, 4), 1.0, None, Alu.mult)
            wts(w3(P2W), bc(p2, 4), 1.0, None, Alu.mult)

            wtt(A[:], DXW[:], VCT4[:], Alu.mult)                   # dx*vct
            wtt(Bv[:], DYW[:], VST4[:], Alu.mult)                  # dy*vst
            wtt(A[:], A[:], Bv[:], Alu.add)                        # A = bd
            wtt(Bv[:], DYW[:], VCT4[:], Alu.mult)
            wtt(C[:], DXW[:], VST4[:], Alu.mult)
            wtt(G1W[:], Bv[:], C[:], Alu.subtract)                 # g1
            wtt(w3(Bv), DX3, bc(ct[:], 4), Alu.mult, bcast=True)
            wtt(w3(C), DY3, bc(st[:], 4), Alu.mult, bcast=True)
            wtt(G2W[:], Bv[:], C[:], Alu.add)                      # g2

            wsq(Bv[:], DXW[:])
            wsq(C[:], DYW[:])
            wtt(Bv[:], Bv[:], C[:], Alu.add)                       # dx2+dy2
            for k in range(4):                                     # barrier
                wts(slab(Bv, k), slab(Bv, k), 1.0, -_f(r2[k]),
                    Alu.mult, Alu.add)
            # DXW/DYW (wa0, wa1) now free
            E1, E2 = wa[0], wa[1]
            wstt(w3(E1), bc(sab[:], 4), 8.0, w3(A), Alu.mult, Alu.mult)
            wstt(w3(E2), bc(smm[:], 4), 16.0, w3(Bv), Alu.mult, Alu.mult)
            wtt(E1[:], E1[:], E2[:], Alu.add)
            wtt(w3(HQW), w3(E1), bc(h0[:], 4), Alu.add, bcast=True)
            wts(HTW[:], HQW[:], 0.5, _f(TOL * 0.5), Alu.mult, Alu.add)

            wsq(A[:], G1W[:])
            wsq(Bv[:], G2W[:])
            wtt(QW[:], A[:], Bv[:], Alu.add)                       # q = gg/4

            wtt(A[:], G1W[:], P1W[:], Alu.mult)
            wtt(Bv[:], G2W[:], P2W[:], Alu.mult)
            wtt(A[:], A[:], Bv[:], Alu.add)                        # A = t_k
            wtt(M0W[:], HTW[:], A[:], Alu.subtract)                # margin0

            # S_jk = g1j g1k + g2j g2k (6 off-diagonal planes)
            pairs = [(0, 1), (0, 2), (0, 3), (1, 2), (1, 3), (2, 3)]
            S = {}
            for (i, j) in pairs:
                a1 = em_t.tmp()
                wtt(a1[:], slab(G1W, i), slab(G1W, j), Alu.mult)
                a2 = em_t.tmp()
                wtt(a2[:], slab(G2W, i), slab(G2W, j), Alu.mult)
                sij = em.plane(f"S{i}{j}")
                wtt(sij[:], a1[:], a2[:], Alu.add)
                S[(i, j)] = S[(j, i)] = sij[:]
            for k in range(4):
                S[(k, k)] = slab(QW, k)

            # ---------------- candidate 0: u0 = -p ----------------
            fmin0 = em_t.tmp()
            wtt(fmin0[:], slab(M0W, 0), slab(M0W, 1), Alu.min)
            f23 = em_t.tmp()
            wtt(f23[:], slab(M0W, 2), slab(M0W, 3), Alu.min)
            wtt(fmin0[:], fmin0[:], f23[:], Alu.min)
            flag0 = em_t.tmp()
            wts(flag0[:], fmin0[:], 0.0, None, Alu.is_ge)

            pp1 = em_t.tmp(); wsq(pp1[:], p1)
            pp2 = em_t.tmp(); wsq(pp2[:], p2)
            pps = em_t.tmp(); wtt(pps[:], pp1[:], pp2[:], Alu.add)

            bo = em.plane("best_obj")
            bx = em.plane("best_ux")
            by = em.plane("best_uy")
            pen = em_t.tmp()
            wts(pen[:], flag0[:], -1e30, 1e30, Alu.mult, Alu.add)
            obj0 = em_t.tmp()
            wts(obj0[:], pps[:], -0.5, None, Alu.mult)
            wtt(bo[:], obj0[:], pen[:], Alu.add)
            wts(bx[:], p1, -1.0, None, Alu.mult)
            wts(by[:], p2, -1.0, None, Alu.mult)

            def fold_candidate(objm, ux, uy):
                bt = em.mask("bt")
                nc.vector.tensor_tensor(bt[:], objm, bo[:], Alu.is_lt)
                nc.vector.copy_predicated(bx[:], bt[:], ux)
                nc.vector.copy_predicated(by[:], bt[:], uy)
                nc.vector.tensor_tensor(bo[:], objm, bo[:], Alu.min)
                eng_load["v"] += 4 * (157 + PC / 0.96)

            # ---------------- u1 candidates, wide over k ----------------
            # A holds t_k from above. LAM2W = (2 t - hq) / (2 q + eps/2)
            U1XW, U1YW, LAM2W = named("U1XW"), named("U1YW"), named("LAM2W")
            wts(Bv[:], QW[:], 2.0, 5e-10, Alu.mult, Alu.add)
            wrecip(C[:], Bv[:])                                    # rden
            wstt(Bv[:], A[:], 2.0, HQW[:], Alu.mult, Alu.subtract)
            wtt(LAM2W[:], Bv[:], C[:], Alu.mult)                   # lam2

            wtt(U1XW[:], LAM2W[:], G1W[:], Alu.mult)
            wtt(U1XW[:], U1XW[:], P1W[:], Alu.subtract)
            wtt(U1YW[:], LAM2W[:], G2W[:], Alu.mult)
            wtt(U1YW[:], U1YW[:], P2W[:], Alu.subtract)

            wts(A[:], LAM2W[:], -_f(2 * TOL), None, Alu.is_ge)     # dual flag

            # feasibility (S-factored, j != k skipped by construction) per k
            MKW = Bv
            for k in range(4):
                fm = None
                for j in range(4):
                    if j == k:
                        continue
                    e = em_t.tmp()
                    wtt(e[:], slab(LAM2W, k), S[(j, k)], Alu.mult)
                    mg = em_t.tmp()
                    wtt(mg[:], e[:], slab(M0W, j), Alu.add)
                    if fm is None:
                        fm = mg
                    else:
                        wtt(fm[:], fm[:], mg[:], Alu.min)
                ff = em_t.tmp()
                wts(ff[:], fm[:], 0.0, None, Alu.is_ge)
                wtt(slab(MKW, k), slab(A, k), ff[:], Alu.mult)

            wsq(C[:], U1XW[:])
            wsq(D[:], U1YW[:])
            wtt(C[:], C[:], D[:], Alu.add)                         # |u|^2
            wtt(D[:], U1XW[:], P1W[:], Alu.mult)
            wtt(A[:], U1YW[:], P2W[:], Alu.mult)
            wtt(D[:], D[:], A[:], Alu.add)                         # p.u
            wstt(C[:], C[:], 0.5, D[:], Alu.mult, Alu.add)         # obj
            wts(D[:], MKW[:], -1e30, 1e30, Alu.mult, Alu.add)
            wtt(C[:], C[:], D[:], Alu.add)                         # objm
            for k in range(4):
                fold_candidate(slab(C, k), slab(U1XW, k), slab(U1YW, k))

            # ---------------- u2 candidates, wide over 6 pairs -----------
            # per-pair products into wide slabs; elementwise chains wide.
            U2XW, U2YW = named("U2XW", 6), named("U2YW", 6)
            Wd, We, Wf, Wg, Wh, Wi = (w[:] for w in wb)

            for pi, (i, j) in enumerate(pairs):                    # det/4
                t1 = em_t.tmp()
                wtt(t1[:], slab(G1W, i), slab(G2W, j), Alu.mult)
                t2 = em_t.tmp()
                wtt(t2[:], slab(G2W, i), slab(G1W, j), Alu.mult)
                wtt(slab(Wd, pi), t1[:], t2[:], Alu.subtract)
            oki = pp.tile([128, W6], U8, tag="oki", name="oki")
            wact(We, Wd, Act.Abs)
            wts(oki[:], We, 2.5e-10, None, Alu.is_gt)
            wts(We, We, 2.5e-10, None, Alu.is_gt)                  # okf (f32)
            nc.vector.memset(Wf, 0.25)
            nc.vector.copy_predicated(Wf, oki[:], Wd)
            eng_load["v"] += 2 * (157 + W6 / 0.96)
            wrecip(Wd, Wf)                                         # rds = 1/ds

            for pi, (i, j) in enumerate(pairs):
                t1 = em_t.tmp()
                wtt(t1[:], slab(HQW, i), slab(G2W, j), Alu.mult)
                t2 = em_t.tmp()
                wtt(t2[:], slab(HQW, j), slab(G2W, i), Alu.mult)
                wtt(slab(Wf, pi), t1[:], t2[:], Alu.subtract)      # e
                t3 = em_t.tmp()
                wtt(t3[:], slab(G1W, j), slab(HQW, i), Alu.mult)
                t4 = em_t.tmp()
                wtt(t4[:], slab(G1W, i), slab(HQW, j), Alu.mult)
                wtt(slab(Wg, pi), t3[:], t4[:], Alu.subtract)      # e2
            wstt(U2XW[:], Wf, -0.5, Wd, Alu.mult, Alu.mult)
            wstt(U2YW[:], Wg, 0.5, Wd, Alu.mult, Alu.mult)

            wtt(w3(Wf, 6)[:], w3(U2XW, 6), bc(p1, 6), Alu.add, bcast=True)
            wtt(w3(Wg, 6)[:], w3(U2YW, 6), bc(p2, 6), Alu.add, bcast=True)
            # Wf = w0 = u2x + p1 ; Wg = w1 = u2y + p2 ; r = -w
            for pi, (i, j) in enumerate(pairs):
                t1 = em_t.tmp()
                wtt(t1[:], slab(Wf, pi), slab(G2W, j), Alu.mult)
                t2 = em_t.tmp()
                wtt(t2[:], slab(Wg, pi), slab(G1W, j), Alu.mult)
                wtt(slab(Wh, pi), t1[:], t2[:], Alu.subtract)      # li pre
                t3 = em_t.tmp()
                wtt(t3[:], slab(G1W, i), slab(Wg, pi), Alu.mult)
                t4 = em_t.tmp()
                wtt(t4[:], slab(G2W, i), slab(Wf, pi), Alu.mult)
                wtt(slab(Wi, pi), t3[:], t4[:], Alu.subtract)      # lj pre
            wstt(Wh, Wh, 0.5, Wd, Alu.mult, Alu.mult)              # li
            wstt(Wi, Wi, 0.5, Wd, Alu.mult, Alu.mult)              # lj
            wts(Wh, Wh, -_f(TOL), None, Alu.is_ge)
            wts(Wi, Wi, -_f(TOL), None, Alu.is_ge)
            wtt(Wh, Wh, Wi, Alu.mult)
            wtt(Wh, Wh, We, Alu.mult)                              # dual2&ok

            # primal feasibility at the two non-active constraints
            for pi, (i, j) in enumerate(pairs):
                fm2 = None
                for m in range(4):
                    if m == i or m == j:
                        continue
                    t1 = em_t.tmp()
                    wtt(t1[:], slab(G1W, m), slab(U2XW, pi), Alu.mult)
                    t2 = em_t.tmp()
                    wtt(t2[:], slab(G2W, m), slab(U2YW, pi), Alu.mult)
                    wtt(t1[:], t1[:], t2[:], Alu.add)
                    mg = em_t.tmp()
                    wtt(mg[:], t1[:], slab(HTW, m), Alu.add)
                    if fm2 is None:
                        fm2 = mg
                    else:
                        wtt(fm2[:], fm2[:], mg[:], Alu.min)
                ff2 = em_t.tmp()
                wts(ff2[:], fm2[:], 0.0, None, Alu.is_ge)
                wtt(slab(Wi, pi), slab(Wh, pi), ff2[:], Alu.mult)  # mask

            wsq(Wd, U2XW[:])
            wsq(We, U2YW[:])
            wtt(Wd, Wd, We, Alu.add)                               # |u|^2
            wtt(w3(We, 6)[:], w3(U2XW, 6), bc(p1, 6), Alu.mult, bcast=True)
            wtt(w3(Wf, 6)[:], w3(U2YW, 6), bc(p2, 6), Alu.mult, bcast=True)
            wtt(We, We, Wf, Alu.add)                               # p.u
            wstt(Wd, Wd, 0.5, We, Alu.mult, Alu.add)               # obj
            wts(We, Wi, -1e30, 1e30, Alu.mult, Alu.add)
            wtt(Wd, Wd, We, Alu.add)                               # objm
            for pi in range(6):
                fold_candidate(slab(Wd, pi), slab(U2XW, pi), slab(U2YW, pi))

            # ---------------- output ----------------
            outsb = wp.tile([128, PC * 2], F32, tag="outsb")
            o3 = outsb[:].rearrange("p (c two) -> p c two", two=2)
            nc.vector.tensor_copy(o3[:, :, 0], bx[:])
            nc.vector.tensor_copy(o3[:, :, 1], by[:])
            nc.sync.dma_start(
                out=out_d.ap().rearrange("(p c) two -> p (c two)", p=128),
                in_=outsb[:])
    nc.compile()
    return nc


def _host_prep(inputs):
    """Fold std/mean into W1/b1; build lhsT weight layouts and consts."""
    mean = np.asarray(inputs["mean"], np.float32)
    std = np.asarray(inputs["std"], np.float32)
    W1 = np.asarray(inputs["W1"], np.float32)
    b1 = np.asarray(inputs["b1"], np.float32)
    W21 = np.asarray(inputs["W21"], np.float32)
    b21 = np.asarray(inputs["b21"], np.float32)
    W22 = np.asarray(inputs["W22"], np.float32)
    b22 = np.asarray(inputs["b22"], np.float32)
    W31 = np.asarray(inputs["W31"], np.float32)
    b31 = np.asarray(inputs["b31"], np.float32)
    W32 = np.asarray(inputs["W32"], np.float32)
    b32 = np.asarray(inputs["b32"], np.float32)

    # NOTE: the reference MLP consumes RAW x (x0 = x*std+mean feeds only the
    # physical-state features), so W1/b1 are used as-is.
    b1p = b1
    w1t = np.ascontiguousarray(W1.T).astype(np.float32)            # [8, 256]

    Wcat = np.vstack([W21, W22]).astype(np.float32)                # [256, 256]
    wcat = np.concatenate([Wcat[:, :128].T, Wcat[:, 128:].T],
                          axis=1)                                  # [128, 512]
    wcat = np.ascontiguousarray(wcat, dtype=np.float32)
    bcat = np.concatenate([b21, b22]).astype(np.float32)

    W3blk = np.zeros((4, 256), np.float32)
    W3blk[0:2, 0:128] = W31
    W3blk[2:4, 128:256] = W32
    w3t = np.concatenate([W3blk[:, :128].T, W3blk[:, 128:].T],
                         axis=1)                                   # [128, 8]
    w3t = np.ascontiguousarray(w3t, dtype=np.float32)
    b3 = np.concatenate([b31, b32]).astype(np.float32)

    consts = np.zeros((128, 16), np.float32)
    consts[:, 0] = b1p[:128]
    consts[:, 1] = b1p[128:]
    consts[:, 2] = bcat[:128]
    consts[:, 3] = bcat[128:]
    consts[:, 4] = b3[3]      # b32[1] (sigmoid bias for s2)
    consts[:, 13] = b3[0]     # b31[0]
    consts[:, 14] = b3[1]     # b31[1]
    consts[:, 15] = b3[2]     # b32[0] (sigmoid bias for s1)
    for k in range(3):
        consts[:, 5 + k] = mean[0] - STATIC_OBS[k, 0]   # dx bias
        consts[:, 8 + k] = mean[1] - STATIC_OBS[k, 1]   # dy bias
    consts[:, 11] = mean[4] - mean[0]                   # oxo bias
    consts[:, 12] = mean[5] - mean[1]                   # oyo bias

    ident = np.eye(128, dtype=np.float32)
    return w1t, wcat, w3t, ident, consts


def kernel(**inputs):
    x = np.ascontiguousarray(np.asarray(inputs["x"], np.float32))
    assert x.shape == (B, NF)
    w1t, wcat, w3t, ident, consts = _host_prep(inputs)

    zb = (not np.any(np.asarray(inputs["b1"]))
          and not np.any(np.asarray(inputs["b21"]))
          and not np.any(np.asarray(inputs["b22"])))
    key = ("nc", zb)
    if key not in _NC_CACHE:
        _NC_CACHE[key] = _build_nc(zero_bias=zb)
    nc = _NC_CACHE[key]

    in_maps = []
    for c in range(NCORES):
        in_maps.append({
            "x": x[c * SHARD:(c + 1) * SHARD],
            "w1t": w1t, "wcat": wcat, "w3t": w3t,
            "ident": ident, "consts": consts,
        })
    res = run_bass_kernel_spmd(nc, in_maps, list(range(NCORES)))
    out = np.concatenate([res.results[c]["out"] for c in range(NCORES)], axis=0)
    return out.astype(np.float32)



# revision 12
# speedup vs baseline: 1.0341x; 1.0341x over previous
"""BarrierNet Trainium2 kernel: MLP + batched 2-var QP (active-set enumeration).

Self-contained: shards B=262144 samples across 8 NeuronCores (data parallel),
runs a Bass/Tile kernel per core, gathers the full output.

Layout per core (SHARD = 32768 samples):
  - MLP runs in 64 tiles of 512 samples, feature-major on the PE
    (fp32r matmuls; x transposed via PE; relu+bias fused in ACT epilogues).
  - L3 outputs (p1, p2, z1, z2) are staged as rows to DRAM, then reloaded as
    sample-major "planes" [128, 256] (sample s = p*256 + c).
  - The QP stage (constraint build + 11-candidate enumeration + argmin) runs
    on fp32 planes across DVE / GPSIMD / ACT.
"""
import numpy as np

import concourse.bass as bass
import concourse.bacc as bacc
import concourse.tile as tile
from concourse import mybir
from concourse.bass_utils import run_bass_kernel_spmd

F32 = mybir.dt.float32
F32R = mybir.dt.float32r
U8 = mybir.dt.uint8
Alu = mybir.AluOpType
Act = mybir.ActivationFunctionType

NCORES = 8
B = 262144
SHARD = B // NCORES            # 32768
PC = SHARD // 128              # 256  (plane free dim)
TILE_N = 512
NT = SHARD // TILE_N           # 64
NF, H1, H2 = 8, 256, 128

# Constants of the nn.Module (not inputs)
STATIC_OBS = np.array([[8.0, -8.0, 1.0], [-9.0, 7.0, 1.0], [10.0, 10.0, 1.5]],
                      np.float32)
AGENT_R, SAFETY = np.float32(0.5), np.float32(0.1)
TOL = 1e-6

_NC_CACHE = {}


def _f(x):
    return float(np.float32(x))


# ---------------------------------------------------------------------------
# Plane-op emission helpers: round-robin 2-input ops over DVE/GPSIMD.
class Emit:
    def __init__(self, nc, pool):
        self.nc = nc
        self.pool = pool
        self._rr = 0
        self._tmp = 0

    def plane(self, tag):
        return self.pool.tile([128, PC], F32, tag=tag, name=tag)

    def tmp(self):
        self._tmp += 1
        t = f"tmp{self._tmp % 9}"
        return self.pool.tile([128, PC], F32, tag=t, name=t, bufs=2)

    def mask(self, tag):
        return self.pool.tile([128, PC], U8, tag=tag, name=tag, bufs=2)

    _CMP = (Alu.is_ge, Alu.is_gt, Alu.is_le, Alu.is_lt)

    def _eng(self, force=None):
        if force is not None:
            return force
        self._rr += 1
        # 1:1 vector:gpsimd for 2-input ops
        return self.nc.gpsimd if self._rr % 2 == 0 else self.nc.vector

    _POOL_OK = (Alu.add, Alu.subtract, Alu.mult)

    def tt(self, out, a, b, op, eng=None):
        if op not in self._POOL_OK:
            eng = self.nc.vector
        self._eng(eng).tensor_tensor(out, a, b, op)
        return out

    def stt(self, out, a, s, b, op0, op1, eng=None):
        # TensorScalarPtr is not supported on Pool
        self.nc.vector.scalar_tensor_tensor(out, a, _f(s), b, op0, op1)
        return out

    def ts(self, out, a, s1, s2, op0, op1=None, eng=None):
        e = self.nc.vector if eng is None else eng
        s1 = s1 if isinstance(s1, bass.AP) else _f(s1)
        if s2 is None:
            e.tensor_scalar(out, a, s1, None, op0)
        else:
            s2 = s2 if isinstance(s2, bass.AP) else _f(s2)
            e.tensor_scalar(out, a, s1, s2, op0, op1)
        return out

    def sq(self, out, a):
        # square via Pool TT (offloads the scalar engine)
        self.nc.gpsimd.tensor_tensor(out, a, a, Alu.mult)
        return out

    def act(self, out, a, func, bias=0.0, scale=1.0):
        self.nc.scalar.activation(out, a, func, bias=bias, scale=scale)
        return out


def _build_nc(zero_bias=False):
    nc = bacc.Bacc("TRN2", target_bir_lowering=False, debug=False,
                   num_devices=NCORES)

    x_d = nc.dram_tensor("x", [SHARD, NF], F32, kind="ExternalInput")
    xt_d = nc.dram_tensor("xt", [NF, SHARD], F32R, kind="ExternalInput")
    w1t_d = nc.dram_tensor("w1t", [NF, H1], F32R, kind="ExternalInput")
    wcat_d = nc.dram_tensor("wcat", [128, 512], F32R, kind="ExternalInput")
    w3t_d = nc.dram_tensor("w3t", [128, 8], F32R, kind="ExternalInput")
    consts_d = nc.dram_tensor("consts", [128, 16], F32, kind="ExternalInput")
    out_d = nc.dram_tensor("out", [SHARD, 2], F32, kind="ExternalOutput")

    with tile.TileContext(nc) as tc:
        with tc.tile_pool(name="wpool", bufs=1) as wp, \
             tc.tile_pool(name="mlp", bufs=2) as mp, \
             tc.tile_pool(name="planes", bufs=1) as pp, \
             tc.tile_pool(name="tmps", bufs=2) as tp, \
             tc.tile_pool(name="psum", bufs=1, space="PSUM") as ps, \
             tc.tile_pool(name="psum2", bufs=1, space="PSUM") as ps2, \
             tc.tile_pool(name="dram", bufs=1, space="DRAM") as dp:

            em = Emit(nc, pp)
            em_t = Emit(nc, tp)

            # ---------------- constants / weights ----------------
            cs = wp.tile([128, 16], F32, tag="consts")
            nc.sync.dma_start(out=cs[:], in_=consts_d.ap())

            # weights DMA'd directly as f32r (bit-identical to f32)
            w1t_r = wp.tile([NF, H1], F32R, tag="w1tr")
            wcat_r = wp.tile([128, 2 * 256], F32R, tag="wcatr")
            w3t_r = wp.tile([128, 2 * 4], F32R, tag="w3tr")
            nc.sync.dma_start(out=w1t_r[:], in_=w1t_d.ap())
            nc.sync.dma_start(out=wcat_r[:], in_=wcat_d.ap())
            nc.sync.dma_start(out=w3t_r[:], in_=w3t_d.ap())

            # xbig: planes view of x  [128, (c=256, f=8)]
            xbig = wp.tile([128, PC * NF], F32, tag="xbig")
            nc.sync.dma_start(
                out=xbig[:], in_=x_d.ap().rearrange("(p c) f -> p (c f)", p=128))
            xb3 = xbig[:].rearrange("p (c f) -> p c f", f=NF)

            def xf(i):  # feature-i plane view, strided
                return xb3[:, :, i]

            # z-row staging in DRAM
            z3_dram = dp.tile([4, SHARD], F32, tag="z3d")

            # ---------------- MLP over 64 tiles ----------------
            for t in range(NT):
                xT_r = mp.tile([NF, TILE_N], F32R, tag="xTr")
                nc.sync.dma_start(
                    out=xT_r[:], in_=xt_d.ap()[:, t * TILE_N:(t + 1) * TILE_N])

                # L1: h1[mc] = relu(W1[mc] @ xT + b1[mc])  (K=8)
                # both m-chunks land in one 2-bank PSUM tile; single fused
                # bias+relu+f32r-round epilogue [128, 1024] on ACT
                h1r = mp.tile([128, 2 * TILE_N], F32R, tag="h1r")
                h1_ps = ps.tile([128, 2 * TILE_N], F32, tag="h1ps", bufs=1)
                for mc in range(2):
                    nc.tensor.matmul(
                        h1_ps[:, mc * TILE_N:(mc + 1) * TILE_N],
                        w1t_r[:, mc * 128:(mc + 1) * 128],
                        xT_r[:], start=True, stop=True)
                if zero_bias:
                    nc.scalar.activation(h1r[:], h1_ps[:], Act.Relu)
                else:
                    nc.scalar.activation(h1r[:, 0:TILE_N], h1_ps[:, 0:TILE_N],
                                         Act.Relu, bias=cs[:, 0:1], scale=1.0)
                    nc.vector.tensor_scalar(h1r[:, TILE_N:], h1_ps[:, TILE_N:],
                                            cs[:, 1:2], 0.0, Alu.add, Alu.max)

                # L2: h2[mo] = relu(sum_kc WcatT[kc][:,mo] @ h1r[kc] + bcat[mo])
                h2r = mp.tile([128, 2 * TILE_N], F32R, tag="h2r")
                h2_ps = ps.tile([128, 2 * TILE_N], F32, tag="h2ps")
                for mo in range(2):
                    for kc in range(2):
                        nc.tensor.matmul(
                            h2_ps[:, mo * TILE_N:(mo + 1) * TILE_N],
                            wcat_r[:, kc * 256 + mo * 128:kc * 256 + (mo + 1) * 128],
                            h1r[:, kc * TILE_N:(kc + 1) * TILE_N],
                            start=(kc == 0), stop=(kc == 1))
                if zero_bias:
                    nc.vector.tensor_scalar(h2r[:], h2_ps[:], 0.0, None,
                                            Alu.max)
                else:
                    nc.scalar.activation(h2r[:, 0:TILE_N], h2_ps[:, 0:TILE_N],
                                         Act.Relu, bias=cs[:, 2:3], scale=1.0)
                    nc.vector.tensor_scalar(h2r[:, TILE_N:], h2_ps[:, TILE_N:],
                                            cs[:, 3:4], 0.0, Alu.add, Alu.max)

                # L3: z3 = W3blk @ [x21; x22] + b3   ([4, 512])
                z3_ps = ps2.tile([4, TILE_N], F32, tag="z3ps", bufs=2)
                for kc in range(2):
                    nc.tensor.matmul(z3_ps[:],
                                     w3t_r[:, kc * 4:(kc + 1) * 4],
                                     h2r[:, kc * TILE_N:(kc + 1) * TILE_N],
                                     start=(kc == 0), stop=(kc == 1))
                # raw z3 -> SBUF -> DRAM; b3 is folded into the plane reload
                z3_sb = mp.tile([4, TILE_N], F32, tag="z3sb")
                if t % 2 == 0:
                    nc.vector.tensor_copy(z3_sb[:], z3_ps[:])
                else:
                    nc.scalar.activation(z3_sb[:], z3_ps[:], Act.Copy)
                nc.sync.dma_start(
                    out=z3_dram[:, t * TILE_N:(t + 1) * TILE_N], in_=z3_sb[:])

            # ============ QP stage on sample-major planes ============
            # Per-k quantities in wide tiles [128, 4*PC] (k-major slabs);
            # per-pair in [128, 6*PC]. Step-0 broadcast APs (DVE only) map
            # per-sample planes across slabs. Wide scratch buffers are
            # allocated once and reused with explicit live ranges.
            W4, W6 = 4 * PC, 6 * PC

            # --- greedy engine balancer (ns cost estimates) ---
            eng_load = {"v": 0.0, "g": 0.0, "a": 0.0}

            def _pick(costs):
                e = min(costs, key=lambda k: eng_load[k] + costs[k])
                eng_load[e] += costs[e]
                return e

            def _fd(ap):
                return ap.free_size()

            def wtt(out, a, b, op, bcast=False):
                fd = _fd(out)
                if bcast or op not in Emit._POOL_OK:
                    eng_load["v"] += 157 + fd / 0.96
                    nc.vector.tensor_tensor(out, a, b, op)
                    return out
                e = _pick({"v": (157 + fd / 0.96) * 1.55,
                           "g": 220 + fd * 2.17})
                (nc.vector if e == "v" else nc.gpsimd).tensor_tensor(out, a, b, op)
                return out

            def wsq(out, a):
                fd = _fd(out)
                e = _pick({"v": (157 + fd / 0.96) * 1.55,
                           "g": 220 + fd * 2.17,
                           "a": 187 + fd / 1.2})
                if e == "a":
                    nc.scalar.activation(out, a, Act.Square)
                else:
                    (nc.vector if e == "v" else nc.gpsimd).tensor_tensor(
                        out, a, a, Alu.mult)
                return out

            def wts(out, a, s1, s2, op0, op1=None):
                eng_load["v"] += 157 + _fd(out) / 1.92
                s1 = s1 if isinstance(s1, bass.AP) else _f(s1)
                if s2 is None:
                    nc.vector.tensor_scalar(out, a, s1, None, op0)
                else:
                    s2 = s2 if isinstance(s2, bass.AP) else _f(s2)
                    nc.vector.tensor_scalar(out, a, s1, s2, op0, op1)
                return out

            def wstt(out, a, s, b, op0, op1):
                eng_load["v"] += 157 + _fd(out) / 0.96
                nc.vector.scalar_tensor_tensor(out, a, _f(s), b, op0, op1)
                return out

            def wact(out, a, func, bias=0.0, scale=1.0):
                eng_load["a"] += 187 + _fd(out) / 1.2
                nc.scalar.activation(out, a, func, bias=bias, scale=scale)
                return out

            def wrecip(out, a):
                eng_load["v"] += 157 + _fd(out) / 0.96
                nc.vector.reciprocal_approx_fast(out, a)
                return out

            def named(tag, n=4):
                return pp.tile([128, n * PC], F32, tag=tag, name=tag)

            def slab(w, i):
                return w[:, i * PC:(i + 1) * PC]

            def bc(plane_ap, n):
                return plane_ap.rearrange(
                    "p (o c) -> p o c", o=1).to_broadcast((128, n, PC))

            def w3(w, n=4):
                return w[:].rearrange("p (o c) -> p o c", o=n)

            # explicit wide scratch buffers
            wa = [named(f"wa{i}") for i in range(6)]          # [128, 4*PC]
            wb = [named(f"wb{i}", 6) for i in range(6)]       # [128, 6*PC]

            # ---------------- planes: reload z rows ----------------
            # b3 = [b31; b32] folding: p = z + b31; sigmoid bias adds b32.
            zpl = []
            for i in range(4):
                z = em_t.tmp()
                nc.sync.dma_start(
                    out=z[:],
                    in_=z3_dram[i, :].rearrange("(p c) -> p c", p=128))
                zpl.append(z)
            zr1, zr2, zs1, zs2 = (z[:] for z in zpl)
            p1t = em.plane("p1")
            wact(p1t[:], zr1, Act.Identity, bias=cs[:, 13:14])
            p2t = em.plane("p2")
            wact(p2t[:], zr2, Act.Identity, bias=cs[:, 14:15])
            p1, p2 = p1t[:], p2t[:]

            # ---------------- geometry (x-derived), wide over k ----------
            mu = [0.0, 0.0, 0.0, 1.0, 6.0, 6.0]
            sg = [1.0, 1.0, 0.5, 0.3, 1.0, 1.0]
            rtot = np.concatenate(
                [AGENT_R + STATIC_OBS[:, 2] + SAFETY,
                 np.array([2 * AGENT_R + SAFETY], np.float32)]).astype(np.float32)
            r2 = (rtot * rtot).astype(np.float32)

            DXW, DYW = wa[0], wa[1]
            for k in range(3):
                wts(slab(DXW, k), xf(0), sg[0], cs[:, 5 + k:6 + k],
                    Alu.mult, Alu.add)
                wts(slab(DYW, k), xf(1), sg[1], cs[:, 8 + k:9 + k],
                    Alu.mult, Alu.add)
            oxo = em_t.tmp()
            wts(oxo[:], xf(4), sg[4], cs[:, 11:12], Alu.mult, Alu.add)
            oyo = em_t.tmp()
            wts(oyo[:], xf(5), sg[5], cs[:, 12:13], Alu.mult, Alu.add)
            wstt(slab(DXW, 3), xf(0), sg[0], oxo[:], Alu.mult, Alu.subtract)
            wstt(slab(DYW, 3), xf(1), sg[1], oyo[:], Alu.mult, Alu.subtract)

            st = em.plane("st")
            wact(st[:], xf(2), Act.Sin, bias=0.0, scale=sg[2])
            sh = em_t.tmp()
            wact(sh[:], xf(2), Act.Sin, bias=0.0, scale=sg[2] * 0.5)
            sh2 = em_t.tmp()
            wsq(sh2[:], sh[:])
            ct = em.plane("ct")
            wts(ct[:], sh2[:], -2.0, 1.0, Alu.mult, Alu.add)

            v = em.plane("v")
            wts(v[:], xf(3), sg[3], float(mu[3]), Alu.mult, Alu.add)
            vst = em.plane("vst")
            wtt(vst[:], v[:], st[:], Alu.mult)
            vct = em.plane("vct")
            wtt(vct[:], v[:], ct[:], Alu.mult)
            h0 = em.plane("H0")
            wstt(h0[:], v[:], 2.0, v[:], Alu.mult, Alu.mult)   # 2*v^2

            sg1 = em.plane("sg1")
            wact(sg1[:], zs1, Act.Sigmoid, bias=cs[:, 15:16])
            sg2p = em.plane("sg2")
            wact(sg2p[:], zs2, Act.Sigmoid, bias=cs[:, 4:5])
            sab = em.plane("sab")
            wtt(sab[:], sg1[:], sg2p[:], Alu.add)
            smm = em.plane("smm")
            wtt(smm[:], sg1[:], sg2p[:], Alu.mult)

            DX3, DY3 = w3(DXW), w3(DYW)
            G1W, G2W = named("G1W"), named("G2W")
            HQW, HTW = named("HQW"), named("HTW")
            QW, M0W = named("QW"), named("M0W")
            A, Bv, C, D = wa[2], wa[3], wa[4], wa[5]

            # materialized broadcast wides (enable Pool for the products)
            VCT4, VST4 = named("VCT4"), named("VST4")
            P1W, P2W = named("P1W"), named("P2W")
            wts(w3(VCT4), bc(vct[:], 4), 1.0, None, Alu.mult)
            wts(w3(VST4), bc(vst[:], 4), 1.0, None, Alu.mult)
            wts(w3(P1W), bc(p1, 4), 1.0, None, Alu.mult)
            wts(w3(P2W), bc(p2, 4), 1.0, None, Alu.mult)

            wtt(A[:], DXW[:], VCT4[:], Alu.mult)                   # dx*vct
            wtt(Bv[:], DYW[:], VST4[:], Alu.mult)                  # dy*vst
            wtt(A[:], A[:], Bv[:], Alu.add)                        # A = bd
            wtt(Bv[:], DYW[:], VCT4[:], Alu.mult)
            wtt(C[:], DXW[:], VST4[:], Alu.mult)
            wtt(G1W[:], Bv[:], C[:], Alu.subtract)                 # g1
            wtt(w3(Bv), DX3, bc(ct[:], 4), Alu.mult, bcast=True)
            wtt(w3(C), DY3, bc(st[:], 4), Alu.mult, bcast=True)
            wtt(G2W[:], Bv[:], C[:], Alu.add)                      # g2

            wsq(Bv[:], DXW[:])
            wsq(C[:], DYW[:])
            wtt(Bv[:], Bv[:], C[:], Alu.add)                       # dx2+dy2
            for k in range(4):                                     # barrier
                wts(slab(Bv, k), slab(Bv, k), 1.0, -_f(r2[k]),
                    Alu.mult, Alu.add)
            # DXW/DYW (wa0, wa1) now free
            E1, E2 = wa[0], wa[1]
            wstt(w3(E1), bc(sab[:], 4), 8.0, w3(A), Alu.mult, Alu.mult)
            wstt(w3(E2), bc(smm[:], 4), 16.0, w3(Bv), Alu.mult, Alu.mult)
            wtt(E1[:], E1[:], E2[:], Alu.add)
            wtt(w3(HQW), w3(E1), bc(h0[:], 4), Alu.add, bcast=True)
            wts(HTW[:], HQW[:], 0.5, _f(TOL * 0.5), Alu.mult, Alu.add)

            wsq(A[:], G1W[:])
            wsq(Bv[:], G2W[:])
            wtt(QW[:], A[:], Bv[:], Alu.add)                       # q = gg/4

            wtt(A[:], G1W[:], P1W[:], Alu.mult)
            wtt(Bv[:], G2W[:], P2W[:], Alu.mult)
            wtt(A[:], A[:], Bv[:], Alu.add)                        # A = t_k
            wtt(M0W[:], HTW[:], A[:], Alu.subtract)                # margin0

            # S_jk = g1j g1k + g2j g2k (6 off-diagonal planes)
            pairs = [(0, 1), (0, 2), (0, 3), (1, 2), (1, 3), (2, 3)]
            S = {}
            for (i, j) in pairs:
                a1 = em_t.tmp()
                wtt(a1[:], slab(G1W, i), slab(G1W, j), Alu.mult)
                a2 = em_t.tmp()
                wtt(a2[:], slab(G2W, i), slab(G2W, j), Alu.mult)
                sij = em.plane(f"S{i}{j}")
                wtt(sij[:], a1[:], a2[:], Alu.add)
                S[(i, j)] = S[(j, i)] = sij[:]
            for k in range(4):
                S[(k, k)] = slab(QW, k)

            # ---------------- candidate 0: u0 = -p ----------------
            fmin0 = em_t.tmp()
            wtt(fmin0[:], slab(M0W, 0), slab(M0W, 1), Alu.min)
            f23 = em_t.tmp()
            wtt(f23[:], slab(M0W, 2), slab(M0W, 3), Alu.min)
            wtt(fmin0[:], fmin0[:], f23[:], Alu.min)
            flag0 = em_t.tmp()
            wts(flag0[:], fmin0[:], 0.0, None, Alu.is_ge)

            # Objective trick: every candidate u satisfies
            #   obj(u) = -0.5|p|^2 + 0.5|u - u0|^2   (u0 = -p),
            # so candidates compare on E = |u - u0|^2 (E0 = 0).
            bo = em.plane("best_obj")
            bx = em.plane("best_ux")
            by = em.plane("best_uy")
            wts(bo[:], flag0[:], -1e30, 1e30, Alu.mult, Alu.add)
            wts(bx[:], p1, -1.0, None, Alu.mult)
            wts(by[:], p2, -1.0, None, Alu.mult)

            def fold_candidate(objm, ux, uy):
                bt = em.mask("bt")
                nc.vector.tensor_tensor(bt[:], objm, bo[:], Alu.is_lt)
                nc.vector.copy_predicated(bx[:], bt[:], ux)
                nc.vector.copy_predicated(by[:], bt[:], uy)
                nc.vector.tensor_tensor(bo[:], objm, bo[:], Alu.min)
                eng_load["v"] += 4 * (157 + PC / 0.96)

            # ---------------- u1 candidates, wide over k ----------------
            # A holds t_k from above. LAM2W = (2 t - hq) / (2 q + eps/2)
            U1XW, U1YW, LAM2W = named("U1XW"), named("U1YW"), named("LAM2W")
            wts(Bv[:], QW[:], 2.0, 5e-10, Alu.mult, Alu.add)
            wrecip(C[:], Bv[:])                                    # rden
            wstt(Bv[:], A[:], 2.0, HQW[:], Alu.mult, Alu.subtract)
            wtt(LAM2W[:], Bv[:], C[:], Alu.mult)                   # lam2

            wtt(U1XW[:], LAM2W[:], G1W[:], Alu.mult)
            wtt(U1XW[:], U1XW[:], P1W[:], Alu.subtract)
            wtt(U1YW[:], LAM2W[:], G2W[:], Alu.mult)
            wtt(U1YW[:], U1YW[:], P2W[:], Alu.subtract)

            wts(A[:], LAM2W[:], -_f(2 * TOL), None, Alu.is_ge)     # dual flag

            # feasibility (S-factored, j != k skipped by construction) per k
            MKW = Bv
            for k in range(4):
                fm = None
                for j in range(4):
                    if j == k:
                        continue
                    e = em_t.tmp()
                    wtt(e[:], slab(LAM2W, k), S[(j, k)], Alu.mult)
                    mg = em_t.tmp()
                    wtt(mg[:], e[:], slab(M0W, j), Alu.add)
                    if fm is None:
                        fm = mg
                    else:
                        wtt(fm[:], fm[:], mg[:], Alu.min)
                ff = em_t.tmp()
                wts(ff[:], fm[:], 0.0, None, Alu.is_ge)
                wtt(slab(MKW, k), slab(A, k), ff[:], Alu.mult)

            # E = |u1 - u0|^2 = lam2^2 * q
            wsq(C[:], LAM2W[:])
            wtt(C[:], C[:], QW[:], Alu.mult)
            wts(D[:], MKW[:], -1e30, 1e30, Alu.mult, Alu.add)
            wtt(C[:], C[:], D[:], Alu.add)                         # E masked
            for k in range(4):
                fold_candidate(slab(C, k), slab(U1XW, k), slab(U1YW, k))

            # ---------------- u2 candidates, wide over 6 pairs -----------
            # per-pair products into wide slabs; elementwise chains wide.
            U2XW, U2YW = named("U2XW", 6), named("U2YW", 6)
            Wd, We, Wf, Wg, Wh, Wi = (w[:] for w in wb)

            for pi, (i, j) in enumerate(pairs):                    # det/4
                t1 = em_t.tmp()
                wtt(t1[:], slab(G1W, i), slab(G2W, j), Alu.mult)
                t2 = em_t.tmp()
                wtt(t2[:], slab(G2W, i), slab(G1W, j), Alu.mult)
                wtt(slab(Wd, pi), t1[:], t2[:], Alu.subtract)
            oki = pp.tile([128, W6], U8, tag="oki", name="oki")
            wact(We, Wd, Act.Abs)
            wts(oki[:], We, 2.5e-10, None, Alu.is_gt)
            wts(We, We, 2.5e-10, None, Alu.is_gt)                  # okf (f32)
            nc.vector.memset(Wf, 0.25)
            nc.vector.copy_predicated(Wf, oki[:], Wd)
            eng_load["v"] += 2 * (157 + W6 / 0.96)
            wrecip(Wd, Wf)                                         # rds = 1/ds

            for pi, (i, j) in enumerate(pairs):
                t1 = em_t.tmp()
                wtt(t1[:], slab(HQW, i), slab(G2W, j), Alu.mult)
                t2 = em_t.tmp()
                wtt(t2[:], slab(HQW, j), slab(G2W, i), Alu.mult)
                wtt(slab(Wf, pi), t1[:], t2[:], Alu.subtract)      # e
                t3 = em_t.tmp()
                wtt(t3[:], slab(G1W, j), slab(HQW, i), Alu.mult)
                t4 = em_t.tmp()
                wtt(t4[:], slab(G1W, i), slab(HQW, j), Alu.mult)
                wtt(slab(Wg, pi), t3[:], t4[:], Alu.subtract)      # e2
            wstt(U2XW[:], Wf, -0.5, Wd, Alu.mult, Alu.mult)
            wstt(U2YW[:], Wg, 0.5, Wd, Alu.mult, Alu.mult)

            wtt(w3(Wf, 6)[:], w3(U2XW, 6), bc(p1, 6), Alu.add, bcast=True)
            wtt(w3(Wg, 6)[:], w3(U2YW, 6), bc(p2, 6), Alu.add, bcast=True)
            # Wf = w0 = u2x + p1 ; Wg = w1 = u2y + p2 ; r = -w
            for pi, (i, j) in enumerate(pairs):
                t1 = em_t.tmp()
                wtt(t1[:], slab(Wf, pi), slab(G2W, j), Alu.mult)
                t2 = em_t.tmp()
                wtt(t2[:], slab(Wg, pi), slab(G1W, j), Alu.mult)
                wtt(slab(Wh, pi), t1[:], t2[:], Alu.subtract)      # li pre
                t3 = em_t.tmp()
                wtt(t3[:], slab(G1W, i), slab(Wg, pi), Alu.mult)
                t4 = em_t.tmp()
                wtt(t4[:], slab(G2W, i), slab(Wf, pi), Alu.mult)
                wtt(slab(Wi, pi), t3[:], t4[:], Alu.subtract)      # lj pre
            wstt(Wh, Wh, 0.5, Wd, Alu.mult, Alu.mult)              # li
            wstt(Wi, Wi, 0.5, Wd, Alu.mult, Alu.mult)              # lj
            wts(Wh, Wh, -_f(TOL), None, Alu.is_ge)
            wts(Wi, Wi, -_f(TOL), None, Alu.is_ge)
            wtt(Wh, Wh, Wi, Alu.mult)
            wtt(Wh, Wh, We, Alu.mult)                              # dual2&ok

            # primal feasibility at the two non-active constraints
            for pi, (i, j) in enumerate(pairs):
                fm2 = None
                for m in range(4):
                    if m == i or m == j:
                        continue
                    t1 = em_t.tmp()
                    wtt(t1[:], slab(G1W, m), slab(U2XW, pi), Alu.mult)
                    t2 = em_t.tmp()
                    wtt(t2[:], slab(G2W, m), slab(U2YW, pi), Alu.mult)
                    wtt(t1[:], t1[:], t2[:], Alu.add)
                    mg = em_t.tmp()
                    wtt(mg[:], t1[:], slab(HTW, m), Alu.add)
                    if fm2 is None:
                        fm2 = mg
                    else:
                        wtt(fm2[:], fm2[:], mg[:], Alu.min)
                ff2 = em_t.tmp()
                wts(ff2[:], fm2[:], 0.0, None, Alu.is_ge)
                wtt(slab(Wi, pi), slab(Wh, pi), ff2[:], Alu.mult)  # mask

            # E = |u2 - u0|^2 = w0^2 + w1^2  (Wf = w0, Wg = w1 still live)
            wsq(Wd, Wf)
            wsq(We, Wg)
            wtt(Wd, Wd, We, Alu.add)                               # E
            wts(We, Wi, -1e30, 1e30, Alu.mult, Alu.add)
            wtt(Wd, Wd, We, Alu.add)                               # E masked
            for pi in range(6):
                fold_candidate(slab(Wd, pi), slab(U2XW, pi), slab(U2YW, pi))

            # ---------------- output ----------------
            outsb = wp.tile([128, PC * 2], F32, tag="outsb")
            o3 = outsb[:].rearrange("p (c two) -> p c two", two=2)
            nc.vector.tensor_copy(o3[:, :, 0], bx[:])
            nc.vector.tensor_copy(o3[:, :, 1], by[:])
            nc.sync.dma_start(
                out=out_d.ap().rearrange("(p c) two -> p (c two)", p=128),
                in_=outsb[:])
    nc.compile()
    return nc


def _host_prep(inputs):
    """Fold std/mean into W1/b1; build lhsT weight layouts and consts."""
    mean = np.asarray(inputs["mean"], np.float32)
    std = np.asarray(inputs["std"], np.float32)
    W1 = np.asarray(inputs["W1"], np.float32)
    b1 = np.asarray(inputs["b1"], np.float32)
    W21 = np.asarray(inputs["W21"], np.float32)
    b21 = np.asarray(inputs["b21"], np.float32)
    W22 = np.asarray(inputs["W22"], np.float32)
    b22 = np.asarray(inputs["b22"], np.float32)
    W31 = np.asarray(inputs["W31"], np.float32)
    b31 = np.asarray(inputs["b31"], np.float32)
    W32 = np.asarray(inputs["W32"], np.float32)
    b32 = np.asarray(inputs["b32"], np.float32)

    # NOTE: the reference MLP consumes RAW x (x0 = x*std+mean feeds only the
    # physical-state features), so W1/b1 are used as-is.
    b1p = b1
    w1t = np.ascontiguousarray(W1.T).astype(np.float32)            # [8, 256]

    Wcat = np.vstack([W21, W22]).astype(np.float32)                # [256, 256]
    wcat = np.concatenate([Wcat[:, :128].T, Wcat[:, 128:].T],
                          axis=1)                                  # [128, 512]
    wcat = np.ascontiguousarray(wcat, dtype=np.float32)
    bcat = np.concatenate([b21, b22]).astype(np.float32)

    W3blk = np.zeros((4, 256), np.float32)
    W3blk[0:2, 0:128] = W31
    W3blk[2:4, 128:256] = W32
    w3t = np.concatenate([W3blk[:, :128].T, W3blk[:, 128:].T],
                         axis=1)                                   # [128, 8]
    w3t = np.ascontiguousarray(w3t, dtype=np.float32)
    b3 = np.concatenate([b31, b32]).astype(np.float32)

    consts = np.zeros((128, 16), np.float32)
    consts[:, 0] = b1p[:128]
    consts[:, 1] = b1p[128:]
    consts[:, 2] = bcat[:128]
    consts[:, 3] = bcat[128:]
    consts[:, 4] = b3[3]      # b32[1] (sigmoid bias for s2)
    consts[:, 13] = b3[0]     # b31[0]
    consts[:, 14] = b3[1]     # b31[1]
    consts[:, 15] = b3[2]     # b32[0] (sigmoid bias for s1)
    for k in range(3):
        consts[:, 5 + k] = mean[0] - STATIC_OBS[k, 0]   # dx bias
        consts[:, 8 + k] = mean[1] - STATIC_OBS[k, 1]   # dy bias
    consts[:, 11] = mean[4] - mean[0]                   # oxo bias
    consts[:, 12] = mean[5] - mean[1]                   # oyo bias

    return w1t, wcat, w3t, consts


def kernel(**inputs):
    x = np.ascontiguousarray(np.asarray(inputs["x"], np.float32))
    assert x.shape == (B, NF)
    w1t, wcat, w3t, consts = _host_prep(inputs)

    zb = (not np.any(np.asarray(inputs["b1"]))
          and not np.any(np.asarray(inputs["b21"]))
          and not np.any(np.asarray(inputs["b22"])))
    key = ("nc", zb)
    if key not in _NC_CACHE:
        _NC_CACHE[key] = _build_nc(zero_bias=zb)
    nc = _NC_CACHE[key]

    in_maps = []
    for c in range(NCORES):
        xs = x[c * SHARD:(c + 1) * SHARD]
        in_maps.append({
            "x": xs,
            "xt": np.ascontiguousarray(xs.T),
            "w1t": w1t, "wcat": wcat, "w3t": w3t,
            "consts": consts,
        })
    res = run_bass_kernel_spmd(nc, in_maps, list(range(NCORES)))
    out = np.concatenate([res.results[c]["out"] for c in range(NCORES)], axis=0)
    return out.astype(np.float32)

